# revision 25
# baseline (speedup 1.0000x reference)
"""Trainium2 Bass kernel for nn_CrossAttentionModule_bias.

Math (B=2, C=256, H=W=64, N=4096):
    q = queries.reshape(B,C,N).T + q_pos        # [B,N,C]
    k = keys.reshape(B,C,N).T + k_pos
    v = values.reshape(B,C,N).T
    attn = softmax(q @ k.T / sqrt(C)) + c_b     # c_b: per-batch SCALAR
    out  = attn @ v   -> [B,C,H,W]

where c_b = softplus(bias_eye*s_eye) + softplus(bias_mouth*s_mouth), s_x =
sum(m*m) over the nearest-resized mask.  Adding the scalar c_b to every attn
entry adds c_b*S[c] (S = colsum V) to every output row; folded host-side into
V: v'[m,c] = v[m,c] + c_b*S[c] makes u'/Z = u/Z + c_b*S[c] exact.

Device kernel (per core, 8 cores = 2 batches x 4 query-column shards):
    dotsT[m,n] = sum_c keff[c,m] * qeff[c,n]    (bf16 matmuls, fp32 PSUM)
    e = exp(dotsT * 1/16) -> bf16               (no max subtraction, |dots|<18)
    U_T[c,n] accumulated over m-chunks on PE
    zsum[p,n] += e[p,n] on DVE (two independent chains to hide RMW latency)
    Zbc = ones128 @ (zsumA + zsumB) -> [128,n] broadcast of Z in one matmul
    out[c,n] = U_T[c,n] * recip(Zbc)[c,n]       (c_b*S folded into vaug)

Timing-critical structure: the QK pipeline runs `lookahead` steps ahead of
AV so PE never waits on ACT's exp latency; tails are deferred two steps so
PE doesn't stall on the DVE reciprocal; the For_i body holds `unroll` reps
to amortize the loop's all-engine barrier + pipeline fill/drain.
"""

import numpy as np
import ml_dtypes

import concourse.bass as bass
import concourse.mybir as mybir
import concourse.tile as tile
from concourse import bacc
from concourse.bass_utils import run_bass_kernel_spmd

# Problem shape (hardcoded per the task contract)
B, C, H, W = 2, 256, 64, 64
N = H * W                      # 4096
NCORES = 8
SHARDS_PER_B = NCORES // B     # 4 query-column shards per batch
NSH = N // SHARDS_PER_B        # 1024 query columns per core
SCALE = float(C) ** -0.5       # 1/16
P = 128
CCN = C // P                   # 2 c-chunks
MCN = N // P                   # 32 m-chunks
NT_SIZE = 512                  # n-tile width (PSUM bank width in fp32)
NTN = NSH // NT_SIZE           # 2 n-tiles per core

F32 = mybir.dt.float32
F32R = mybir.dt.float32r
BF16 = mybir.dt.bfloat16

EXP = mybir.ActivationFunctionType.Exp

_CACHE: dict = {}


def _build_bass(reps: int = 1, loop_reps: int = 0, ablate: tuple = (),
                unroll: int = 4, lookahead: int = 2):
    """reps>1 unrolls the whole compute; loop_reps>0 wraps it in a hardware
    For_i loop (timing-only variants: slope between two loop_reps builds
    isolates per-iteration HW time from the ~100ms dispatch floor).  The
    For_i body holds `unroll` reps per iteration."""
    nc = bacc.Bacc("TRN2", target_bir_lowering=False, debug=False)

    keff = nc.dram_tensor("keff", [C, N], BF16, kind="ExternalInput")
    qeff = nc.dram_tensor("qeff", [C, NSH], BF16, kind="ExternalInput")
    vaug = nc.dram_tensor("vaug", [N, C], BF16, kind="ExternalInput")
    out = nc.dram_tensor("out", [C, NSH], F32, kind="ExternalOutput")

    KQ = 8                     # m-chunks per keff DMA tile
    KQN = MCN // KQ            # 4 keff tiles per c-chunk

    with tile.TileContext(nc) as tc:
        with (
            tc.tile_pool(name="const", bufs=1) as cpool,
            tc.tile_pool(name="work", bufs=4) as wpool,
            tc.tile_pool(name="zs", bufs=2) as zpool,
            tc.tile_pool(name="tail", bufs=3) as tpool,
            tc.tile_pool(name="dots_ps", bufs=2, space="PSUM") as dots_pool,
            tc.tile_pool(name="acc_ps", bufs=1, space="PSUM") as acc_pool,
        ):
            zero = cpool.tile([P, 1], F32, tag="zero", name="zero")
            nc.vector.memset(zero[:], 0.0)
            # all-ones [128,128] stationary: one matmul turns colsum+broadcast
            # of zsum into Zbc[j,n] = Z[n] on every partition j.  f32r matmul
            # operands must be produced as f32r; memset can't, so stage via copy.
            ones_f = cpool.tile([P, P], F32, tag="ones_f", name="ones_f")
            nc.vector.memset(ones_f[:], 1.0)
            ones128 = cpool.tile([P, P], F32R, tag="ones128", name="ones128")
            with nc.allow_low_precision(reason="f32r ones are exact"):
                nc.vector.tensor_copy(ones128[:], ones_f[:])

            qeff_t = []
            for cc in range(CCN):
                t = cpool.tile([P, NSH], BF16, tag=f"qeff{cc}", name=f"qeff{cc}")
                nc.sync.dma_start(t[:], qeff[cc * P : (cc + 1) * P, :])
                qeff_t.append(t)

            # keff split into [128, KQ*128] tiles so QK can start early
            keff_t = [[None] * KQN for _ in range(CCN)]
            for q in range(KQN):
                for cc in range(CCN):
                    t = cpool.tile([P, KQ * P], BF16, tag=f"keff{cc}_{q}", name=f"keff{cc}_{q}")
                    nc.sync.dma_start(
                        t[:], keff[cc * P : (cc + 1) * P, q * KQ * P : (q + 1) * KQ * P]
                    )
                    keff_t[cc][q] = t

            vaug_t = []
            for mc in range(MCN):
                t = cpool.tile([P, C], BF16, tag=f"vaug{mc}", name=f"vaug{mc}")
                nc.sync.dma_start(t[:], vaug[mc * P : (mc + 1) * P, :])
                vaug_t.append(t)

            const_expt = None
            if "exp" in ablate:
                const_expt = cpool.tile([P, 2 * NT_SIZE], BF16, tag="cexpt", name="cexpt")
                nc.vector.memset(const_expt[:], 1.0)

            def emit_qk(nt, mp):
                # one [128,1024] 2-bank pair: m-chunks (2*mp, 2*mp+1) of this
                # n-tile, so a single ACT exp covers both (halves the ACT
                # instruction count)
                ns = slice(nt * NT_SIZE, (nt + 1) * NT_SIZE)
                dots = dots_pool.tile([P, 2 * NT_SIZE], F32, tag="dots", name="dots")
                for j in range(2):
                    mc = 2 * mp + j
                    for cc in range(CCN):
                        lhsT = keff_t[cc][mc // KQ][:, (mc % KQ) * P : (mc % KQ + 1) * P]
                        nc.tensor.matmul(
                            dots[:, j * NT_SIZE : (j + 1) * NT_SIZE],
                            lhsT,
                            qeff_t[cc][:, ns],
                            start=(cc == 0),
                            stop=(cc == CCN - 1),
                        )
                return dots

            def emit_body(nt):
                # one n-tile per body: consecutive bodies alternate n-tiles,
                # so body b+1's AV accumulators are different PSUM banks and
                # body b's tail (Zbc/recip/muls) overlaps b+1's m-loop
                ns = slice(nt * NT_SIZE, (nt + 1) * NT_SIZE)
                u_ps = [
                    acc_pool.tile([P, NT_SIZE], F32, tag=f"u{cc}n{nt}", name=f"u{cc}n{nt}")
                    for cc in range(CCN)
                ]
                # two independent [128,512] chains (pair parity): independent
                # chains hide the DVE read-modify-write latency
                zsum = [None, None]

                MPN = MCN // 2
                dots_q = [emit_qk(nt, 0)]
                for mp in range(MPN):
                    dots = dots_q.pop(0)
                    if "exp" in ablate:
                        expt = const_expt
                    else:
                        expt = wpool.tile([P, 2 * NT_SIZE], BF16, tag="expt", name="expt")
                        nc.scalar.activation(expt[:], dots[:], EXP, bias=zero[:], scale=SCALE)
                    if mp + 1 < MPN:
                        dots_q.append(emit_qk(nt, mp + 1))
                    for j in range(2):
                        mc = 2 * mp + j
                        for cc in range(CCN):
                            nc.tensor.matmul(
                                u_ps[cc][:],
                                vaug_t[mc][:, cc * P : (cc + 1) * P],
                                expt[:, j * NT_SIZE : (j + 1) * NT_SIZE],
                                start=(mc == 0),
                                stop=(mc == MCN - 1),
                            )
                    if "z" not in ablate:
                        ch = mp % 2
                        # f32r tag so the colsum matmul reads "rounded" input
                        with nc.allow_low_precision(reason="zsum feeds f32r matmul"):
                            for j in range(2):
                                js = slice(j * NT_SIZE, (j + 1) * NT_SIZE)
                                if mp < 2:
                                    if j == 0:
                                        zsum[ch] = zpool.tile(
                                            [P, NT_SIZE], F32R,
                                            tag=f"zsum{ch}", name=f"zsum{ch}",
                                        )
                                        nc.vector.tensor_copy(zsum[ch][:], expt[:, js])
                                        continue
                                nc.vector.tensor_add(zsum[ch][:], zsum[ch][:], expt[:, js])

                if "tail" in ablate or "z" in ablate:
                    return
                # tail: Z colsum + partition-broadcast in a matmul pair
                # (borrowing half a dots pool slot for PSUM), then one
                # reciprocal and one multiply per c-chunk.  Runs concurrently
                # with the next body's m-loop (different u banks there).
                zbc_pair = dots_pool.tile([P, 2 * NT_SIZE], F32, tag="dots", name="zbc_pair")
                zbc = zbc_pair[:, 0:NT_SIZE]
                nc.tensor.matmul(zbc, ones128[:], zsum[0][:], start=True, stop=False)
                nc.tensor.matmul(zbc, ones128[:], zsum[1][:], start=False, stop=True)
                recip = tpool.tile([P, NT_SIZE], F32, tag="recip", name="recip")
                nc.vector.reciprocal(recip[:], zbc)
                for cc in range(CCN):
                    outsb = tpool.tile([P, NT_SIZE], F32, tag="outsb", name="outsb")
                    nc.vector.tensor_mul(outsb[:], u_ps[cc][:], recip[:])
                    if "dma" not in ablate:
                        nc.sync.dma_start(out[cc * P : (cc + 1) * P, ns], outsb[:])

            if loop_reps > 0:
                # one rep = NTN bodies; u bodies per For_i iteration
                u = unroll
                while u % NTN or (loop_reps * NTN) % u:
                    u -= 1
                with tc.For_i(
                    0, loop_reps * NTN // u, 1, hint_engines=(mybir.EngineType.PE,)
                ):
                    for b in range(u):
                        emit_body(b % NTN)
            else:
                for _ in range(reps):
                    for nt in range(NTN):
                        emit_body(nt)

    nc.compile()
    return nc


def _prep_inputs(queries, keys, values, mask_eye, mask_mouth, q_pos, k_pos,
                 bias_eye, bias_mouth):
    """Host-side shard prep: positional adds, V transpose, the per-batch
    scalar bias folded into V as a constant row add, bf16 conversion."""
    q = queries.reshape(B, C, N) + q_pos[0].T[None]
    k = keys.reshape(B, C, N) + k_pos[0].T[None]
    vT = values.reshape(B, C, N).transpose(0, 2, 1)  # [B,N,C]

    def msum(mask):
        # nearest resize 128->64 picks every other row/col
        m = mask[:, :, ::2, ::2].reshape(B, -1)
        return (m * m).sum(axis=1, dtype=np.float64)

    softplus = lambda x: np.logaddexp(0.0, x)
    c_b = softplus(float(bias_eye[0]) * msum(mask_eye)) + softplus(
        float(bias_mouth[0]) * msum(mask_mouth)
    )  # [B]
    S = vT.sum(axis=1, dtype=np.float64)  # [B, C]
    cbs = c_b[:, None] * S  # [B, C]

    vaug = np.ascontiguousarray(
        (vT.astype(np.float64) + cbs[:, None, :]).astype(ml_dtypes.bfloat16)
    )  # [B,N,C]
    kb = k.astype(ml_dtypes.bfloat16)
    qb = q.astype(ml_dtypes.bfloat16)

    in_maps = []
    for core in range(NCORES):
        b, sh = divmod(core, SHARDS_PER_B)
        n0 = sh * NSH
        in_maps.append(
            {
                "keff": np.ascontiguousarray(kb[b]),
                "qeff": np.ascontiguousarray(qb[b][:, n0 : n0 + NSH]),
                "vaug": vaug[b],
            }
        )
    return in_maps


def kernel(**inputs) -> np.ndarray:
    inputs = {k: np.asarray(v, np.float32) for k, v in inputs.items()}
    in_maps = _prep_inputs(**inputs)

    if "nc" not in _CACHE:
        _CACHE["nc"] = _build_bass()
    res = run_bass_kernel_spmd(_CACHE["nc"], in_maps, list(range(NCORES)))

    full = np.empty((B, C, N), np.float32)
    for core in range(NCORES):
        b, sh = divmod(core, SHARDS_PER_B)
        n0 = sh * NSH
        full[b][:, n0 : n0 + NSH] = res.results[core]["out"]
    return full.reshape(B, C, H, W)


# revision 28
# speedup vs baseline: 1.1535x; 1.1535x over previous
"""Trainium2 Bass kernel for nn_CrossAttentionModule_bias.

Math (B=2, C=256, H=W=64, N=4096):
    q = queries.reshape(B,C,N).T + q_pos        # [B,N,C]
    k = keys.reshape(B,C,N).T + k_pos
    v = values.reshape(B,C,N).T
    attn = softmax(q @ k.T / sqrt(C)) + c_b     # c_b: per-batch SCALAR
    out  = attn @ v   -> [B,C,H,W]

where c_b = softplus(bias_eye*s_eye) + softplus(bias_mouth*s_mouth), s_x =
sum(m*m) over the nearest-resized mask.  Adding the scalar c_b to every attn
entry adds c_b*S[c] (S = colsum V) to every output row; folded host-side into
V: v'[m,c] = v[m,c] + c_b*S[c] makes u'/Z = u/Z + c_b*S[c] exact.

Device kernel (per core, 8 cores = 2 batches x 4 query-column shards):
    dotsT[m,n] = sum_c keff[c,m] * qeff[c,n]    (bf16 matmuls, fp32 PSUM)
    e = exp(dotsT * 1/16) -> bf16               (no max subtraction, |dots|<18)
    U_T[c,n] accumulated over m-chunks on PE
    zsum[p,n] += e[p,n] on DVE (two independent chains to hide RMW latency)
    Zbc = ones128 @ (zsumA + zsumB) -> [128,n] broadcast of Z in one matmul
    out[c,n] = U_T[c,n] * recip(Zbc)[c,n]       (c_b*S folded into vaug)

Timing-critical structure: the QK pipeline runs `lookahead` steps ahead of
AV so PE never waits on ACT's exp latency; tails are deferred two steps so
PE doesn't stall on the DVE reciprocal; the For_i body holds `unroll` reps
to amortize the loop's all-engine barrier + pipeline fill/drain.
"""

import numpy as np
import ml_dtypes

import concourse.bass as bass
import concourse.mybir as mybir
import concourse.tile as tile
from concourse import bacc
from concourse.bass_utils import run_bass_kernel_spmd

# Problem shape (hardcoded per the task contract)
B, C, H, W = 2, 256, 64, 64
N = H * W                      # 4096
NCORES = 8
SHARDS_PER_B = NCORES // B     # 4 query-column shards per batch
NSH = N // SHARDS_PER_B        # 1024 query columns per core
SCALE = float(C) ** -0.5       # 1/16
P = 128
CCN = C // P                   # 2 c-chunks
MCN = N // P                   # 32 m-chunks
NT_SIZE = 512                  # n-tile width (PSUM bank width in fp32)
NTN = NSH // NT_SIZE           # 2 n-tiles per core

F32 = mybir.dt.float32
F32R = mybir.dt.float32r
BF16 = mybir.dt.bfloat16

EXP = mybir.ActivationFunctionType.Exp

_CACHE: dict = {}


def _build_bass(reps: int = 1, loop_reps: int = 0, ablate: tuple = (),
                unroll: int = 4, lookahead: int = 2):
    """reps>1 unrolls the whole compute; loop_reps>0 wraps it in a hardware
    For_i loop (timing-only variants: slope between two loop_reps builds
    isolates per-iteration HW time from the ~100ms dispatch floor).  The
    For_i body holds `unroll` reps per iteration."""
    nc = bacc.Bacc("TRN2", target_bir_lowering=False, debug=False)

    keff = nc.dram_tensor("keff", [C, N], BF16, kind="ExternalInput")
    qeff = nc.dram_tensor("qeff", [C, NSH], BF16, kind="ExternalInput")
    vaug = nc.dram_tensor("vaug", [N, C], BF16, kind="ExternalInput")
    out = nc.dram_tensor("out", [C, NSH], F32, kind="ExternalOutput")

    KQ = 8                     # m-chunks per keff DMA tile
    KQN = MCN // KQ            # 4 keff tiles per c-chunk

    with tile.TileContext(nc) as tc:
        with (
            tc.tile_pool(name="const", bufs=1) as cpool,
            tc.tile_pool(name="work", bufs=4) as wpool,
            tc.tile_pool(name="zs", bufs=2) as zpool,
            tc.tile_pool(name="tail", bufs=3) as tpool,
            tc.tile_pool(name="dots_ps", bufs=2, space="PSUM") as dots_pool,
            tc.tile_pool(name="acc_ps", bufs=1, space="PSUM") as acc_pool,
        ):
            zero = cpool.tile([P, 1], F32, tag="zero", name="zero")
            nc.vector.memset(zero[:], 0.0)
            # all-ones [128,128] stationary: one matmul turns colsum+broadcast
            # of zsum into Zbc[j,n] = Z[n] on every partition j.  f32r matmul
            # operands must be produced as f32r; memset can't, so stage via copy.
            ones_f = cpool.tile([P, P], F32, tag="ones_f", name="ones_f")
            nc.vector.memset(ones_f[:], 1.0)
            ones128 = cpool.tile([P, P], F32R, tag="ones128", name="ones128")
            with nc.allow_low_precision(reason="f32r ones are exact"):
                nc.vector.tensor_copy(ones128[:], ones_f[:])

            qeff_t = []
            for cc in range(CCN):
                t = cpool.tile([P, NSH], BF16, tag=f"qeff{cc}", name=f"qeff{cc}")
                nc.sync.dma_start(t[:], qeff[cc * P : (cc + 1) * P, :])
                qeff_t.append(t)

            # keff split into [128, KQ*128] tiles so QK can start early
            keff_t = [[None] * KQN for _ in range(CCN)]
            for q in range(KQN):
                for cc in range(CCN):
                    t = cpool.tile([P, KQ * P], BF16, tag=f"keff{cc}_{q}", name=f"keff{cc}_{q}")
                    nc.sync.dma_start(
                        t[:], keff[cc * P : (cc + 1) * P, q * KQ * P : (q + 1) * KQ * P]
                    )
                    keff_t[cc][q] = t

            vaug_t = []
            for mc in range(MCN):
                t = cpool.tile([P, C], BF16, tag=f"vaug{mc}", name=f"vaug{mc}")
                nc.sync.dma_start(t[:], vaug[mc * P : (mc + 1) * P, :])
                vaug_t.append(t)

            const_expt = None
            if "exp" in ablate:
                const_expt = cpool.tile([P, 2 * NT_SIZE], BF16, tag="cexpt", name="cexpt")
                nc.vector.memset(const_expt[:], 1.0)

            def emit_qk(nt, mp):
                # one [128,1024] 2-bank pair: m-chunks (2*mp, 2*mp+1) of this
                # n-tile, so a single ACT exp covers both (halves the ACT
                # instruction count)
                ns = slice(nt * NT_SIZE, (nt + 1) * NT_SIZE)
                dots = dots_pool.tile([P, 2 * NT_SIZE], F32, tag="dots", name="dots")
                for j in range(2):
                    mc = 2 * mp + j
                    for cc in range(CCN):
                        lhsT = keff_t[cc][mc // KQ][:, (mc % KQ) * P : (mc % KQ + 1) * P]
                        nc.tensor.matmul(
                            dots[:, j * NT_SIZE : (j + 1) * NT_SIZE],
                            lhsT,
                            qeff_t[cc][:, ns],
                            start=(cc == 0),
                            stop=(cc == CCN - 1),
                        )
                return dots

            def emit_body(nt, pending_tail=None):
                # one n-tile per body: consecutive bodies alternate n-tiles,
                # so body b+1's AV accumulators are different PSUM banks and
                # body b's tail (Zbc/recip/muls) overlaps b+1's m-loop.  The
                # tail is EMITTED inside body b+1 (pending_tail) so the PE
                # doesn't hit the Zbc->zsum wait before starting b+1's QK.
                ns = slice(nt * NT_SIZE, (nt + 1) * NT_SIZE)
                u_ps = [
                    acc_pool.tile([P, NT_SIZE], F32, tag=f"u{cc}n{nt}", name=f"u{cc}n{nt}")
                    for cc in range(CCN)
                ]
                # two independent [128,512] chains (pair parity): independent
                # chains hide the DVE read-modify-write latency
                zsum = [None, None]

                MPN = MCN // 2
                dots_q = [emit_qk(nt, 0)]
                for mp in range(MPN):
                    dots = dots_q.pop(0)
                    if "exp" in ablate:
                        expt = const_expt
                    else:
                        expt = wpool.tile([P, 2 * NT_SIZE], BF16, tag="expt", name="expt")
                        nc.scalar.activation(expt[:], dots[:], EXP, bias=zero[:], scale=SCALE)
                    if mp + 1 < MPN:
                        dots_q.append(emit_qk(nt, mp + 1))
                    for j in range(2):
                        mc = 2 * mp + j
                        for cc in range(CCN):
                            nc.tensor.matmul(
                                u_ps[cc][:],
                                vaug_t[mc][:, cc * P : (cc + 1) * P],
                                expt[:, j * NT_SIZE : (j + 1) * NT_SIZE],
                                start=(mc == 0),
                                stop=(mc == MCN - 1),
                            )
                    if "z" not in ablate:
                        ch = mp % 2
                        # f32r tag so the colsum matmul reads "rounded" input
                        with nc.allow_low_precision(reason="zsum feeds f32r matmul"):
                            for j in range(2):
                                js = slice(j * NT_SIZE, (j + 1) * NT_SIZE)
                                if mp < 2:
                                    if j == 0:
                                        zsum[ch] = zpool.tile(
                                            [P, NT_SIZE], F32R,
                                            tag=f"zsum{ch}", name=f"zsum{ch}",
                                        )
                                        nc.vector.tensor_copy(zsum[ch][:], expt[:, js])
                                        continue
                                nc.vector.tensor_add(zsum[ch][:], zsum[ch][:], expt[:, js])
                    if mp == 1 and pending_tail is not None:
                        pending_tail()
                        pending_tail = None

                if pending_tail is not None:  # ablated bodies don't reach mp==1 path
                    pending_tail()
                if "tail" in ablate or "z" in ablate:
                    return None

                def tail():
                    # tail: Z colsum + partition-broadcast in a matmul pair
                    # (borrowing half a dots pool slot for PSUM), then one
                    # reciprocal and one multiply per c-chunk.  Runs
                    # concurrently with the next body's m-loop (different u
                    # banks there).
                    zbc_pair = dots_pool.tile(
                        [P, 2 * NT_SIZE], F32, tag="dots", name="zbc_pair"
                    )
                    zbc = zbc_pair[:, 0:NT_SIZE]
                    nc.tensor.matmul(zbc, ones128[:], zsum[0][:], start=True, stop=False)
                    nc.tensor.matmul(zbc, ones128[:], zsum[1][:], start=False, stop=True)
                    recip = tpool.tile([P, NT_SIZE], F32, tag="recip", name="recip")
                    nc.vector.reciprocal(recip[:], zbc)
                    for cc in range(CCN):
                        outsb = tpool.tile([P, NT_SIZE], F32, tag="outsb", name="outsb")
                        nc.vector.tensor_mul(outsb[:], u_ps[cc][:], recip[:])
                        if "dma" not in ablate:
                            nc.sync.dma_start(out[cc * P : (cc + 1) * P, ns], outsb[:])

                return tail

            if loop_reps > 0:
                # one rep = NTN bodies; u bodies per For_i iteration
                u = unroll
                while u % NTN or (loop_reps * NTN) % u:
                    u -= 1
                with tc.For_i(
                    0, loop_reps * NTN // u, 1, hint_engines=(mybir.EngineType.PE,)
                ):
                    pending = None
                    for b in range(u):
                        pending = emit_body(b % NTN, pending)
                    if pending is not None:
                        pending()
            else:
                pending = None
                for _ in range(reps):
                    for nt in range(NTN):
                        pending = emit_body(nt, pending)
                if pending is not None:
                    pending()

    nc.compile()
    return nc


def _prep_inputs(queries, keys, values, mask_eye, mask_mouth, q_pos, k_pos,
                 bias_eye, bias_mouth):
    """Host-side shard prep: positional adds, V transpose, the per-batch
    scalar bias folded into V as a constant row add, bf16 conversion."""
    q = queries.reshape(B, C, N) + q_pos[0].T[None]
    k = keys.reshape(B, C, N) + k_pos[0].T[None]
    vT = values.reshape(B, C, N).transpose(0, 2, 1)  # [B,N,C]

    def msum(mask):
        # nearest resize 128->64 picks every other row/col
        m = mask[:, :, ::2, ::2].reshape(B, -1)
        return (m * m).sum(axis=1, dtype=np.float64)

    softplus = lambda x: np.logaddexp(0.0, x)
    c_b = softplus(float(bias_eye[0]) * msum(mask_eye)) + softplus(
        float(bias_mouth[0]) * msum(mask_mouth)
    )  # [B]
    S = vT.sum(axis=1, dtype=np.float64)  # [B, C]
    cbs = c_b[:, None] * S  # [B, C]

    vaug = np.ascontiguousarray(
        (vT.astype(np.float64) + cbs[:, None, :]).astype(ml_dtypes.bfloat16)
    )  # [B,N,C]
    kb = k.astype(ml_dtypes.bfloat16)
    qb = q.astype(ml_dtypes.bfloat16)

    in_maps = []
    for core in range(NCORES):
        b, sh = divmod(core, SHARDS_PER_B)
        n0 = sh * NSH
        in_maps.append(
            {
                "keff": np.ascontiguousarray(kb[b]),
                "qeff": np.ascontiguousarray(qb[b][:, n0 : n0 + NSH]),
                "vaug": vaug[b],
            }
        )
    return in_maps


def kernel(**inputs) -> np.ndarray:
    inputs = {k: np.asarray(v, np.float32) for k, v in inputs.items()}
    in_maps = _prep_inputs(**inputs)

    if "nc" not in _CACHE:
        _CACHE["nc"] = _build_bass()
    res = run_bass_kernel_spmd(_CACHE["nc"], in_maps, list(range(NCORES)))

    full = np.empty((B, C, N), np.float32)
    for core in range(NCORES):
        b, sh = divmod(core, SHARDS_PER_B)
        n0 = sh * NSH
        full[b][:, n0 : n0 + NSH] = res.results[core]["out"]
    return full.reshape(B, C, H, W)


# revision 29
# speedup vs baseline: 1.3448x; 1.1658x over previous
"""Trainium2 Bass kernel for nn_CrossAttentionModule_bias.

Math (B=2, C=256, H=W=64, N=4096):
    q = queries.reshape(B,C,N).T + q_pos        # [B,N,C]
    k = keys.reshape(B,C,N).T + k_pos
    v = values.reshape(B,C,N).T
    attn = softmax(q @ k.T / sqrt(C)) + c_b     # c_b: per-batch SCALAR
    out  = attn @ v   -> [B,C,H,W]

where c_b = softplus(bias_eye*s_eye) + softplus(bias_mouth*s_mouth), s_x =
sum(m*m) over the nearest-resized mask.  Adding the scalar c_b to every attn
entry adds c_b*S[c] (S = colsum V) to every output row; folded host-side into
V: v'[m,c] = v[m,c] + c_b*S[c] makes u'/Z = u/Z + c_b*S[c] exact.

Device kernel (per core, 8 cores = 2 batches x 4 query-column shards):
    dotsT[m,n] = sum_c keff[c,m] * qeff[c,n]    (bf16 matmuls, fp32 PSUM)
    e = exp(dotsT * 1/16) -> bf16               (no max subtraction, |dots|<18)
    U_T[c,n] accumulated over m-chunks on PE
    zsum[p,n] += e[p,n] on DVE (two independent chains to hide RMW latency)
    Zbc = ones128 @ (zsumA + zsumB) -> [128,n] broadcast of Z in one matmul
    out[c,n] = U_T[c,n] * recip(Zbc)[c,n]       (c_b*S folded into vaug)

Timing-critical structure: the QK pipeline runs `lookahead` steps ahead of
AV so PE never waits on ACT's exp latency; tails are deferred two steps so
PE doesn't stall on the DVE reciprocal; the For_i body holds `unroll` reps
to amortize the loop's all-engine barrier + pipeline fill/drain.
"""

import numpy as np
import ml_dtypes

import concourse.bass as bass
import concourse.mybir as mybir
import concourse.tile as tile
from concourse import bacc
from concourse.bass_utils import run_bass_kernel_spmd

# Problem shape (hardcoded per the task contract)
B, C, H, W = 2, 256, 64, 64
N = H * W                      # 4096
NCORES = 8
SHARDS_PER_B = NCORES // B     # 4 query-column shards per batch
NSH = N // SHARDS_PER_B        # 1024 query columns per core
SCALE = float(C) ** -0.5       # 1/16
P = 128
CCN = C // P                   # 2 c-chunks
MCN = N // P                   # 32 m-chunks
NT_SIZE = 512                  # n-tile width (PSUM bank width in fp32)
NTN = NSH // NT_SIZE           # 2 n-tiles per core

F32 = mybir.dt.float32
F32R = mybir.dt.float32r
BF16 = mybir.dt.bfloat16

EXP = mybir.ActivationFunctionType.Exp

_CACHE: dict = {}


def _build_bass(reps: int = 1, loop_reps: int = 0, ablate: tuple = (),
                unroll: int = 4, lookahead: int = 2):
    """reps>1 unrolls the whole compute; loop_reps>0 wraps it in a hardware
    For_i loop (timing-only variants: slope between two loop_reps builds
    isolates per-iteration HW time from the ~100ms dispatch floor).  The
    For_i body holds `unroll` reps per iteration."""
    nc = bacc.Bacc("TRN2", target_bir_lowering=False, debug=False)

    keff = nc.dram_tensor("keff", [C, N], BF16, kind="ExternalInput")
    qeff = nc.dram_tensor("qeff", [C, NSH], BF16, kind="ExternalInput")
    vaug = nc.dram_tensor("vaug", [N, C], BF16, kind="ExternalInput")
    out = nc.dram_tensor("out", [C, NSH], F32, kind="ExternalOutput")

    KQ = 8                     # m-chunks per keff DMA tile
    KQN = MCN // KQ            # 4 keff tiles per c-chunk

    with tile.TileContext(nc) as tc:
        with (
            tc.tile_pool(name="const", bufs=1) as cpool,
            tc.tile_pool(name="work", bufs=6) as wpool,
            tc.tile_pool(name="zs", bufs=2) as zpool,
            tc.tile_pool(name="tail", bufs=3) as tpool,
            tc.tile_pool(name="dots_ps", bufs=3, space="PSUM") as dots_pool,
            tc.tile_pool(name="acc_ps", bufs=1, space="PSUM") as acc_pool,
            tc.tile_pool(name="bc_ps", bufs=1, space="PSUM") as bc_pool,
        ):
            zero = cpool.tile([P, 1], F32, tag="zero", name="zero")
            nc.vector.memset(zero[:], 0.0)
            # all-ones [128,128] stationary: one matmul turns colsum+broadcast
            # of zsum into Zbc[j,n] = Z[n] on every partition j.  f32r matmul
            # operands must be produced as f32r; memset can't, so stage via copy.
            ones_f = cpool.tile([P, P], F32, tag="ones_f", name="ones_f")
            nc.vector.memset(ones_f[:], 1.0)
            ones128 = cpool.tile([P, P], F32R, tag="ones128", name="ones128")
            with nc.allow_low_precision(reason="f32r ones are exact"):
                nc.vector.tensor_copy(ones128[:], ones_f[:])

            qeff_t = []
            for cc in range(CCN):
                t = cpool.tile([P, NSH], BF16, tag=f"qeff{cc}", name=f"qeff{cc}")
                nc.sync.dma_start(t[:], qeff[cc * P : (cc + 1) * P, :])
                qeff_t.append(t)

            # keff split into [128, KQ*128] tiles so QK can start early
            keff_t = [[None] * KQN for _ in range(CCN)]
            for q in range(KQN):
                for cc in range(CCN):
                    t = cpool.tile([P, KQ * P], BF16, tag=f"keff{cc}_{q}", name=f"keff{cc}_{q}")
                    nc.sync.dma_start(
                        t[:], keff[cc * P : (cc + 1) * P, q * KQ * P : (q + 1) * KQ * P]
                    )
                    keff_t[cc][q] = t

            vaug_t = []
            for mc in range(MCN):
                t = cpool.tile([P, C], BF16, tag=f"vaug{mc}", name=f"vaug{mc}")
                nc.sync.dma_start(t[:], vaug[mc * P : (mc + 1) * P, :])
                vaug_t.append(t)

            const_expt = None
            if "exp" in ablate:
                const_expt = cpool.tile([P, NT_SIZE], BF16, tag="cexpt", name="cexpt")
                nc.vector.memset(const_expt[:], 1.0)

            def emit_qk(nt, mc):
                ns = slice(nt * NT_SIZE, (nt + 1) * NT_SIZE)
                dots = dots_pool.tile([P, NT_SIZE], F32, tag="dots", name="dots")
                for cc in range(CCN):
                    lhsT = keff_t[cc][mc // KQ][:, (mc % KQ) * P : (mc % KQ + 1) * P]
                    nc.tensor.matmul(
                        dots[:],
                        lhsT,
                        qeff_t[cc][:, ns],
                        start=(cc == 0),
                        stop=(cc == CCN - 1),
                    )
                return dots

            def emit_body():
                u_ps = [
                    [
                        acc_pool.tile([P, NT_SIZE], F32, tag=f"u{cc}n{nt}", name=f"u{cc}n{nt}")
                        for cc in range(CCN)
                    ]
                    for nt in range(NTN)
                ]
                zsum = [[None, None] for _ in range(NTN)]  # two chains per nt

                def emit_tail(nt):
                    # Z colsum + partition-broadcast in one matmul pair, then
                    # one reciprocal and one multiply per c-chunk.
                    ns = slice(nt * NT_SIZE, (nt + 1) * NT_SIZE)
                    zbc = bc_pool.tile([P, NT_SIZE], F32, tag="zbc", name="zbc")
                    nc.tensor.matmul(zbc[:], ones128[:], zsum[nt][0][:], start=True, stop=False)
                    nc.tensor.matmul(zbc[:], ones128[:], zsum[nt][1][:], start=False, stop=True)
                    recip = tpool.tile([P, NT_SIZE], F32, tag="recip", name="recip")
                    nc.vector.reciprocal(recip[:], zbc[:])
                    for cc in range(CCN):
                        outsb = tpool.tile([P, NT_SIZE], F32, tag="outsb", name="outsb")
                        nc.vector.tensor_mul(outsb[:], u_ps[nt][cc][:], recip[:])
                        nc.sync.dma_start(out[cc * P : (cc + 1) * P, ns], outsb[:])

                # software-pipelined `lookahead` deep; tails deferred 2 steps
                steps = [(nt, mc) for nt in range(NTN) for mc in range(MCN)]
                pending_tail = []
                dots_q = [emit_qk(*steps[j]) for j in range(lookahead)]
                for i, (nt, mc) in enumerate(steps):
                    dots = dots_q.pop(0)
                    if "exp" in ablate:
                        expt = const_expt
                    else:
                        expt = wpool.tile([P, NT_SIZE], BF16, tag="expt", name="expt")
                        nc.scalar.activation(expt[:], dots[:], EXP, bias=zero[:], scale=SCALE)
                    if i + lookahead < len(steps):
                        dots_q.append(emit_qk(*steps[i + lookahead]))
                    first, last = mc == 0, mc == MCN - 1
                    for cc in range(CCN):
                        nc.tensor.matmul(
                            u_ps[nt][cc][:],
                            vaug_t[mc][:, cc * P : (cc + 1) * P],
                            expt[:],
                            start=first,
                            stop=last,
                        )
                    if "z" not in ablate:
                        ch = mc % 2
                        # f32r tag so the colsum matmul reads "rounded" input
                        with nc.allow_low_precision(reason="zsum feeds f32r matmul"):
                            if mc < 2:
                                zsum[nt][ch] = zpool.tile(
                                    [P, NT_SIZE], F32R, tag=f"zsum{ch}", name=f"zsum{ch}"
                                )
                                nc.vector.tensor_copy(zsum[nt][ch][:], expt[:])
                            else:
                                nc.vector.tensor_add(zsum[nt][ch][:], zsum[nt][ch][:], expt[:])
                    if pending_tail and pending_tail[0][0] <= i:
                        emit_tail(pending_tail.pop(0)[1])
                    if last and "tail" not in ablate and "z" not in ablate:
                        pending_tail.append((i + 2, nt))
                for due, nt in pending_tail:
                    emit_tail(nt)

            if loop_reps > 0:
                u = unroll
                while loop_reps % u:
                    u -= 1
                with tc.For_i(0, loop_reps // u, 1, hint_engines=(mybir.EngineType.PE,)):
                    for _ in range(u):
                        emit_body()
            else:
                for _ in range(reps):
                    emit_body()

    nc.compile()
    return nc


def _prep_inputs(queries, keys, values, mask_eye, mask_mouth, q_pos, k_pos,
                 bias_eye, bias_mouth):
    """Host-side shard prep: positional adds, V transpose, the per-batch
    scalar bias folded into V as a constant row add, bf16 conversion."""
    q = queries.reshape(B, C, N) + q_pos[0].T[None]
    k = keys.reshape(B, C, N) + k_pos[0].T[None]
    vT = values.reshape(B, C, N).transpose(0, 2, 1)  # [B,N,C]

    def msum(mask):
        # nearest resize 128->64 picks every other row/col
        m = mask[:, :, ::2, ::2].reshape(B, -1)
        return (m * m).sum(axis=1, dtype=np.float64)

    softplus = lambda x: np.logaddexp(0.0, x)
    c_b = softplus(float(bias_eye[0]) * msum(mask_eye)) + softplus(
        float(bias_mouth[0]) * msum(mask_mouth)
    )  # [B]
    S = vT.sum(axis=1, dtype=np.float64)  # [B, C]
    cbs = c_b[:, None] * S  # [B, C]

    vaug = np.ascontiguousarray(
        (vT.astype(np.float64) + cbs[:, None, :]).astype(ml_dtypes.bfloat16)
    )  # [B,N,C]
    kb = k.astype(ml_dtypes.bfloat16)
    qb = q.astype(ml_dtypes.bfloat16)

    in_maps = []
    for core in range(NCORES):
        b, sh = divmod(core, SHARDS_PER_B)
        n0 = sh * NSH
        in_maps.append(
            {
                "keff": np.ascontiguousarray(kb[b]),
                "qeff": np.ascontiguousarray(qb[b][:, n0 : n0 + NSH]),
                "vaug": vaug[b],
            }
        )
    return in_maps


def kernel(**inputs) -> np.ndarray:
    inputs = {k: np.asarray(v, np.float32) for k, v in inputs.items()}
    in_maps = _prep_inputs(**inputs)

    if "nc" not in _CACHE:
        _CACHE["nc"] = _build_bass()
    res = run_bass_kernel_spmd(_CACHE["nc"], in_maps, list(range(NCORES)))

    full = np.empty((B, C, N), np.float32)
    for core in range(NCORES):
        b, sh = divmod(core, SHARDS_PER_B)
        n0 = sh * NSH
        full[b][:, n0 : n0 + NSH] = res.results[core]["out"]
    return full.reshape(B, C, H, W)


# revision 30
# speedup vs baseline: 1.3762x; 1.0233x over previous
"""Trainium2 Bass kernel for nn_CrossAttentionModule_bias.

Math (B=2, C=256, H=W=64, N=4096):
    q = queries.reshape(B,C,N).T + q_pos        # [B,N,C]
    k = keys.reshape(B,C,N).T + k_pos
    v = values.reshape(B,C,N).T
    attn = softmax(q @ k.T / sqrt(C)) + c_b     # c_b: per-batch SCALAR
    out  = attn @ v   -> [B,C,H,W]

where c_b = softplus(bias_eye*s_eye) + softplus(bias_mouth*s_mouth), s_x =
sum(m*m) over the nearest-resized mask.  Adding the scalar c_b to every attn
entry adds c_b*S[c] (S = colsum V) to every output row; folded host-side into
V: v'[m,c] = v[m,c] + c_b*S[c] makes u'/Z = u/Z + c_b*S[c] exact.

Device kernel (per core, 8 cores = 2 batches x 4 query-column shards):
    dotsT[m,n] = sum_c keff[c,m] * qeff[c,n]    (bf16 matmuls, fp32 PSUM)
    e = exp(dotsT * 1/16) -> bf16               (no max subtraction, |dots|<18)
    U_T[c,n] accumulated over m-chunks on PE
    zsum[p,n] += e[p,n] on DVE (two independent chains to hide RMW latency)
    Zbc = ones128 @ (zsumA + zsumB) -> [128,n] broadcast of Z in one matmul
    out[c,n] = U_T[c,n] * recip(Zbc)[c,n]       (c_b*S folded into vaug)

Timing-critical structure: the QK pipeline runs `lookahead` steps ahead of
AV so PE never waits on ACT's exp latency; tails are deferred two steps so
PE doesn't stall on the DVE reciprocal; the For_i body holds `unroll` reps
to amortize the loop's all-engine barrier + pipeline fill/drain.
"""

import numpy as np
import ml_dtypes

import concourse.bass as bass
import concourse.mybir as mybir
import concourse.tile as tile
from concourse import bacc
from concourse.bass_utils import run_bass_kernel_spmd

# Problem shape (hardcoded per the task contract)
B, C, H, W = 2, 256, 64, 64
N = H * W                      # 4096
NCORES = 8
SHARDS_PER_B = NCORES // B     # 4 query-column shards per batch
NSH = N // SHARDS_PER_B        # 1024 query columns per core
SCALE = float(C) ** -0.5       # 1/16
P = 128
CCN = C // P                   # 2 c-chunks
MCN = N // P                   # 32 m-chunks
NT_SIZE = 512                  # n-tile width (PSUM bank width in fp32)
NTN = NSH // NT_SIZE           # 2 n-tiles per core

F32 = mybir.dt.float32
F32R = mybir.dt.float32r
BF16 = mybir.dt.bfloat16

EXP = mybir.ActivationFunctionType.Exp

_CACHE: dict = {}


def _build_bass(reps: int = 1, loop_reps: int = 0, ablate: tuple = (),
                unroll: int = 8, lookahead: int = 2):
    """reps>1 unrolls the whole compute; loop_reps>0 wraps it in a hardware
    For_i loop (timing-only variants: slope between two loop_reps builds
    isolates per-iteration HW time from the ~100ms dispatch floor).  The
    For_i body holds `unroll` reps per iteration."""
    nc = bacc.Bacc("TRN2", target_bir_lowering=False, debug=False)

    keff = nc.dram_tensor("keff", [C, N], BF16, kind="ExternalInput")
    qeff = nc.dram_tensor("qeff", [C, NSH], BF16, kind="ExternalInput")
    vaug = nc.dram_tensor("vaug", [N, C], BF16, kind="ExternalInput")
    out = nc.dram_tensor("out", [C, NSH], F32, kind="ExternalOutput")

    KQ = 8                     # m-chunks per keff DMA tile
    KQN = MCN // KQ            # 4 keff tiles per c-chunk

    with tile.TileContext(nc) as tc:
        with (
            tc.tile_pool(name="const", bufs=1) as cpool,
            tc.tile_pool(name="work", bufs=6) as wpool,
            tc.tile_pool(name="zs", bufs=2) as zpool,
            tc.tile_pool(name="tail", bufs=3) as tpool,
            tc.tile_pool(name="dots_ps", bufs=3, space="PSUM") as dots_pool,
            tc.tile_pool(name="acc_ps", bufs=1, space="PSUM") as acc_pool,
            tc.tile_pool(name="bc_ps", bufs=1, space="PSUM") as bc_pool,
        ):
            zero = cpool.tile([P, 1], F32, tag="zero", name="zero")
            nc.vector.memset(zero[:], 0.0)
            # all-ones [128,128] stationary: one matmul turns colsum+broadcast
            # of zsum into Zbc[j,n] = Z[n] on every partition j.  f32r matmul
            # operands must be produced as f32r; memset can't, so stage via copy.
            ones_f = cpool.tile([P, P], F32, tag="ones_f", name="ones_f")
            nc.vector.memset(ones_f[:], 1.0)
            ones128 = cpool.tile([P, P], F32R, tag="ones128", name="ones128")
            with nc.allow_low_precision(reason="f32r ones are exact"):
                nc.vector.tensor_copy(ones128[:], ones_f[:])

            qeff_t = []
            for cc in range(CCN):
                t = cpool.tile([P, NSH], BF16, tag=f"qeff{cc}", name=f"qeff{cc}")
                nc.sync.dma_start(t[:], qeff[cc * P : (cc + 1) * P, :])
                qeff_t.append(t)

            # keff split into [128, KQ*128] tiles so QK can start early
            keff_t = [[None] * KQN for _ in range(CCN)]
            for q in range(KQN):
                for cc in range(CCN):
                    t = cpool.tile([P, KQ * P], BF16, tag=f"keff{cc}_{q}", name=f"keff{cc}_{q}")
                    nc.sync.dma_start(
                        t[:], keff[cc * P : (cc + 1) * P, q * KQ * P : (q + 1) * KQ * P]
                    )
                    keff_t[cc][q] = t

            vaug_t = []
            for mc in range(MCN):
                t = cpool.tile([P, C], BF16, tag=f"vaug{mc}", name=f"vaug{mc}")
                nc.sync.dma_start(t[:], vaug[mc * P : (mc + 1) * P, :])
                vaug_t.append(t)

            const_expt = None
            if "exp" in ablate:
                const_expt = cpool.tile([P, NT_SIZE], BF16, tag="cexpt", name="cexpt")
                nc.vector.memset(const_expt[:], 1.0)

            def emit_qk(nt, mc):
                ns = slice(nt * NT_SIZE, (nt + 1) * NT_SIZE)
                dots = dots_pool.tile([P, NT_SIZE], F32, tag="dots", name="dots")
                for cc in range(CCN):
                    lhsT = keff_t[cc][mc // KQ][:, (mc % KQ) * P : (mc % KQ + 1) * P]
                    nc.tensor.matmul(
                        dots[:],
                        lhsT,
                        qeff_t[cc][:, ns],
                        start=(cc == 0),
                        stop=(cc == CCN - 1),
                    )
                return dots

            def emit_body():
                u_ps = [
                    [
                        acc_pool.tile([P, NT_SIZE], F32, tag=f"u{cc}n{nt}", name=f"u{cc}n{nt}")
                        for cc in range(CCN)
                    ]
                    for nt in range(NTN)
                ]
                zsum = [[None, None] for _ in range(NTN)]  # two chains per nt

                def emit_tail(nt):
                    # Z colsum + partition-broadcast in one matmul pair, then
                    # one reciprocal and one multiply per c-chunk.
                    ns = slice(nt * NT_SIZE, (nt + 1) * NT_SIZE)
                    zbc = bc_pool.tile([P, NT_SIZE], F32, tag="zbc", name="zbc")
                    nc.tensor.matmul(zbc[:], ones128[:], zsum[nt][0][:], start=True, stop=False)
                    nc.tensor.matmul(zbc[:], ones128[:], zsum[nt][1][:], start=False, stop=True)
                    recip = tpool.tile([P, NT_SIZE], F32, tag="recip", name="recip")
                    nc.vector.reciprocal(recip[:], zbc[:])
                    for cc in range(CCN):
                        outsb = tpool.tile([P, NT_SIZE], F32, tag="outsb", name="outsb")
                        nc.vector.tensor_mul(outsb[:], u_ps[nt][cc][:], recip[:])
                        nc.sync.dma_start(out[cc * P : (cc + 1) * P, ns], outsb[:])

                # software-pipelined `lookahead` deep; tails deferred 2 steps
                steps = [(nt, mc) for nt in range(NTN) for mc in range(MCN)]
                pending_tail = []
                dots_q = [emit_qk(*steps[j]) for j in range(lookahead)]
                for i, (nt, mc) in enumerate(steps):
                    dots = dots_q.pop(0)
                    if "exp" in ablate:
                        expt = const_expt
                    else:
                        expt = wpool.tile([P, NT_SIZE], BF16, tag="expt", name="expt")
                        nc.scalar.activation(expt[:], dots[:], EXP, bias=zero[:], scale=SCALE)
                    if i + lookahead < len(steps):
                        dots_q.append(emit_qk(*steps[i + lookahead]))
                    first, last = mc == 0, mc == MCN - 1
                    for cc in range(CCN):
                        nc.tensor.matmul(
                            u_ps[nt][cc][:],
                            vaug_t[mc][:, cc * P : (cc + 1) * P],
                            expt[:],
                            start=first,
                            stop=last,
                        )
                    if "z" not in ablate:
                        ch = mc % 2
                        # f32r tag so the colsum matmul reads "rounded" input
                        with nc.allow_low_precision(reason="zsum feeds f32r matmul"):
                            if mc < 2:
                                zsum[nt][ch] = zpool.tile(
                                    [P, NT_SIZE], F32R, tag=f"zsum{ch}", name=f"zsum{ch}"
                                )
                                nc.vector.tensor_copy(zsum[nt][ch][:], expt[:])
                            else:
                                nc.vector.tensor_add(zsum[nt][ch][:], zsum[nt][ch][:], expt[:])
                    if pending_tail and pending_tail[0][0] <= i:
                        emit_tail(pending_tail.pop(0)[1])
                    if last and "tail" not in ablate and "z" not in ablate:
                        pending_tail.append((i + 2, nt))
                for due, nt in pending_tail:
                    emit_tail(nt)

            if loop_reps > 0:
                u = unroll
                while loop_reps % u:
                    u -= 1
                with tc.For_i(0, loop_reps // u, 1, hint_engines=(mybir.EngineType.PE,)):
                    for _ in range(u):
                        emit_body()
            else:
                for _ in range(reps):
                    emit_body()

    nc.compile()
    return nc


def _prep_inputs(queries, keys, values, mask_eye, mask_mouth, q_pos, k_pos,
                 bias_eye, bias_mouth):
    """Host-side shard prep: positional adds, V transpose, the per-batch
    scalar bias folded into V as a constant row add, bf16 conversion."""
    q = queries.reshape(B, C, N) + q_pos[0].T[None]
    k = keys.reshape(B, C, N) + k_pos[0].T[None]
    vT = values.reshape(B, C, N).transpose(0, 2, 1)  # [B,N,C]

    def msum(mask):
        # nearest resize 128->64 picks every other row/col
        m = mask[:, :, ::2, ::2].reshape(B, -1)
        return (m * m).sum(axis=1, dtype=np.float64)

    softplus = lambda x: np.logaddexp(0.0, x)
    c_b = softplus(float(bias_eye[0]) * msum(mask_eye)) + softplus(
        float(bias_mouth[0]) * msum(mask_mouth)
    )  # [B]
    S = vT.sum(axis=1, dtype=np.float64)  # [B, C]
    cbs = c_b[:, None] * S  # [B, C]

    vaug = np.ascontiguousarray(
        (vT.astype(np.float64) + cbs[:, None, :]).astype(ml_dtypes.bfloat16)
    )  # [B,N,C]
    kb = k.astype(ml_dtypes.bfloat16)
    qb = q.astype(ml_dtypes.bfloat16)

    in_maps = []
    for core in range(NCORES):
        b, sh = divmod(core, SHARDS_PER_B)
        n0 = sh * NSH
        in_maps.append(
            {
                "keff": np.ascontiguousarray(kb[b]),
                "qeff": np.ascontiguousarray(qb[b][:, n0 : n0 + NSH]),
                "vaug": vaug[b],
            }
        )
    return in_maps


def kernel(**inputs) -> np.ndarray:
    inputs = {k: np.asarray(v, np.float32) for k, v in inputs.items()}
    in_maps = _prep_inputs(**inputs)

    if "nc" not in _CACHE:
        _CACHE["nc"] = _build_bass()
    res = run_bass_kernel_spmd(_CACHE["nc"], in_maps, list(range(NCORES)))

    full = np.empty((B, C, N), np.float32)
    for core in range(NCORES):
        b, sh = divmod(core, SHARDS_PER_B)
        n0 = sh * NSH
        full[b][:, n0 : n0 + NSH] = res.results[core]["out"]
    return full.reshape(B, C, H, W)
